# revision 21
# baseline (speedup 1.0000x reference)
"""Trainium2 Bass kernel for nn_M02SameVQ (VQ codebook match + refiner MLP).

v4 redesign (8 NeuronCores, data-parallel over batch):
 - Coarse scores s = x.c - 0.5||c||^2 in fp16 on the PE (fp32 PSUM accum),
   bias folded as a K=2 matmul of split-fp16 rows; j-outer loop with a
   4-deep PSUM pool so the PE never stalls on evacuation.
 - Top-8 scan on VectorE (fp16), top-2 candidates fetched exactly via
   per-partition indirect DMA gathers (no DRAM index bounce), re-scored in
   fp32 with a fused tensor_tensor_reduce against a PE-transposed x tile.
 - Winner row selected in-place (predicated copy), cast fp16, and moved
   back to channel-major with a single xbar DMA transpose per slot.
 - Refiner MLP in fp16 at 512-token blocks (125 real tokens per 128-slot),
   one-block software skew so the PE stream stays dense; db2 applied via a
   K=1 ones matmul.
"""

import numpy as np
import concourse.tile as tile
import concourse.mybir as mybir
from concourse import bacc, bass, bass_utils
from concourse.masks import make_identity

F32 = mybir.dt.float32
F16 = mybir.dt.float16
U8 = mybir.dt.uint8
U32 = mybir.dt.uint32
AF = mybir.ActivationFunctionType
ALU = mybir.AluOpType
AX = mybir.AxisListType
LRELU = AF.Lrelu  # swapped to Relu by the CoreSim harness (sim lacks Lrelu)
XBAR_LIN = True   # lin back-transpose via xbar DMA (False: PE transpose)
IND_GATHER = True  # rescue rows via indirect DMA (False: fixed rows, debug)
USE_TTR = False   # tensor_tensor_reduce faults on this HW build; use fallback
USE_C2MM = True   # db2 via K=1 matmul (False: skip; db2 is zero anyway)
PAD_MEMSET = True  # strided pad-column memset on fs16

B, C, T = 16, 1024, 1500
NBINS, HID, EMB = 4096, 512, 256
NCORES = 8
BPC = B // NCORES          # batches per core
NT = 125                   # real tokens per slot
SLOT = 128                 # token slot width (3 pad columns)
NSLOT = 4                  # slots per block
BLKR = NT * NSLOT          # real tokens per block (500)
BLKW = SLOT * NSLOT        # slot columns per block (512)
NBLK = T // BLKR           # blocks per batch (3)
CE = 1088                  # ext codebook row: 1024 cen | -0.5||c||^2 | 1.0 | pad
KG = C // 128              # 8 contraction chunks
NJ = NBINS // 512          # 8 bin-tiles

_CACHE = {}


def _build_body(nc, tc, d):
    with tc.tile_pool(name="const", bufs=1) as cp, \
         tc.tile_pool(name="work", bufs=1) as wp, \
         tc.tile_pool(name="psd", bufs=1, space="PSUM") as psd, \
         tc.tile_pool(name="psx", bufs=1, space="PSUM") as psx, \
         tc.tile_pool(name="psm", bufs=1, space="PSUM") as psm:

        # ---- resident constants ----
        cs16 = cp.tile([128, KG, NBINS], F16)
        nc.sync.dma_start(out=cs16, in_=d["cT16"].rearrange("(g p) b -> p g b", p=128))
        bias2 = cp.tile([2, NBINS], F16)
        nc.sync.dma_start(out=bias2, in_=d["bias2"])
        ones2 = cp.tile([2, SLOT], F16)
        nc.vector.memset(ones2, 1.0)
        onesrow = cp.tile([1, BLKW], F16)
        nc.vector.memset(onesrow, 1.0)
        ident = cp.tile([128, 128], F32)
        make_identity(nc, ident)
        ident16 = cp.tile([128, 128], F16)
        nc.vector.tensor_copy(out=ident16, in_=ident)

        w0 = cp.tile([128, KG, HID], F16)
        nc.sync.dma_start(out=w0, in_=d["w0T"].rearrange("(g p) h -> p g h", p=128))
        w1 = cp.tile([128, 4, HID], F16)
        nc.sync.dma_start(out=w1, in_=d["w1T"].rearrange("(g p) h -> p g h", p=128))
        w2 = cp.tile([128, 4, EMB], F16)
        nc.sync.dma_start(out=w2, in_=d["w2T"].rearrange("(g p) h -> p g h", p=128))
        v0 = cp.tile([128, 2, HID], F16)
        nc.sync.dma_start(out=v0, in_=d["v0T"].rearrange("(g p) h -> p g h", p=128))
        v1 = cp.tile([128, 4, HID], F16)
        nc.sync.dma_start(out=v1, in_=d["v1T"].rearrange("(g p) h -> p g h", p=128))
        v2 = cp.tile([128, 4, C], F16)
        nc.sync.dma_start(out=v2, in_=d["v2T"].rearrange("(g p) h -> p g h", p=128))
        b0 = cp.tile([128, 4], F32)
        nc.sync.dma_start(out=b0, in_=d["b0"].rearrange("(g p) -> p g", p=128))
        b1 = cp.tile([128, 4], F32)
        nc.sync.dma_start(out=b1, in_=d["b1"].rearrange("(g p) -> p g", p=128))
        b2 = cp.tile([128, 2], F32)
        nc.sync.dma_start(out=b2, in_=d["b2"].rearrange("(g p) -> p g", p=128))
        c0 = cp.tile([128, 4], F32)
        nc.sync.dma_start(out=c0, in_=d["c0"].rearrange("(g p) -> p g", p=128))
        c1 = cp.tile([128, 4], F32)
        nc.sync.dma_start(out=c1, in_=d["c1"].rearrange("(g p) -> p g", p=128))
        c2r = cp.tile([1, C], F16)
        nc.sync.dma_start(out=c2r, in_=d["c2r"])

        saved = {}

        def front(bi):
            batch, blk = divmod(bi, NBLK)
            feat_b = d["feat"][batch].rearrange("(g p) t -> p g t", p=128)
            fs16 = wp.tile([128, KG, BLKW], F16, tag="fs16", bufs=2)
            linT = wp.tile([128, KG, BLKW], F16, tag="linT", bufs=2)
            # pad columns (slot cols 125..127) never carry real data
            nc.vector.memset(
                fs16.rearrange("p g (t x) -> p g t x", x=SLOT)[:, :, :, NT:], 0.0)

            state = {}

            def stage_a(t):
                tok0 = blk * BLKR + t * NT
                csl = slice(t * SLOT, t * SLOT + NT)
                fs32t = wp.tile([128, KG, NT], F32, tag="fs32t", bufs=2)
                nc.sync.dma_start(out=fs32t, in_=feat_b[:, :, tok0:tok0 + NT])
                nc.scalar.copy(fs16[:, :, csl], fs32t)

                # coarse fp16 scores (scaled 1/8 on evacuation: finer fp16 ulp)
                s16 = wp.tile([NT, NBINS], F16, tag="s16", bufs=2)
                for j in range(NJ):
                    pj = psd.tile([NT, 512], F32, tag="dist", bufs=4)
                    for g in range(KG):
                        nc.tensor.matmul(pj, lhsT=fs16[:, g, csl],
                                         rhs=cs16[:, g, j * 512:(j + 1) * 512],
                                         start=(g == 0), stop=False)
                    nc.tensor.matmul(pj, lhsT=ones2[:, :NT],
                                     rhs=bias2[:, j * 512:(j + 1) * 512],
                                     start=False, stop=True)
                    nc.scalar.mul(s16[:, j * 512:(j + 1) * 512], pj, 0.125)

                # top-8 scan
                v8 = wp.tile([NT, 8], F16, tag="v8", bufs=2)
                idx8 = wp.tile([NT, 8], U32, tag="idx8", bufs=2)
                nc.vector.max(out=v8, in_=s16)
                nc.vector.max_index(out=idx8, in_max=v8, in_values=s16)

                # x_t via PE transpose, evacuated to SBUF
                xtp = psx.tile([NT, C], F32, tag="xt", bufs=1)
                for g in range(KG):
                    nc.tensor.transpose(xtp[:, g * 128:(g + 1) * 128],
                                        fs32t[:, g, :], ident)
                x32t = wp.tile([NT, C + 1], F32, tag="x32t", bufs=2)
                nc.scalar.copy(x32t[:, :C], xtp)
                nc.vector.memset(x32t[:, C:C + 1], 1.0)
                state[t] = [idx8, x32t]

            def stage_g(t):
                idx8, x32t = state[t]
                gs = []
                for k in range(3):
                    gk = wp.tile([NT, CE], F32, tag=f"g{k}",
                                 bufs=2 if k == 0 else 1)
                    nc.gpsimd.indirect_dma_start(
                        out=gk, out_offset=None, in_=d["cen_ext"],
                        in_offset=bass.IndirectOffsetOnAxis(
                            ap=idx8[:, k:k + 1], axis=0))
                    gs.append(gk)
                state[t] = [gs, x32t]

            def stage_b(t):
                gs, x32t = state.pop(t)
                sex = []
                for k in range(3):
                    pk = wp.tile([NT, C + 1], F32, tag="prod", bufs=1)
                    sk = wp.tile([NT, 1], F32, tag=f"sex{k}", bufs=1)
                    nc.gpsimd.tensor_mul(pk, x32t, gs[k][:, :C + 1])
                    nc.vector.tensor_reduce(sk, pk, AX.X, ALU.add)
                    sex.append(sk)
                # select winner row in place on gs[0]
                mk = wp.tile([NT, 1], U8, tag="mk", bufs=1)
                nc.vector.tensor_tensor(mk, sex[1], sex[0], ALU.is_gt)
                nc.vector.copy_predicated(gs[0][:, :C], mk.to_broadcast([NT, C]),
                                          gs[1][:, :C])
                sm = wp.tile([NT, 1], F32, tag="sm", bufs=1)
                nc.vector.tensor_tensor(sm, sex[1], sex[0], ALU.max)
                mk2 = wp.tile([NT, 1], U8, tag="mk2", bufs=1)
                nc.vector.tensor_tensor(mk2, sex[2], sm, ALU.is_gt)
                nc.vector.copy_predicated(gs[0][:, :C], mk2.to_broadcast([NT, C]),
                                          gs[2][:, :C])
                lin16 = wp.tile([SLOT, C], F16, tag="lin16", bufs=2)
                nc.vector.tensor_copy(out=lin16[:NT], in_=gs[0][:, :C])
                nc.sync.dma_start_transpose(
                    linT[:, :, t * SLOT:(t + 1) * SLOT], lin16)

            # one-slot software skew; gathers issue on gpsimd before the
            # previous slot's rescue multiplies so neither blocks the other
            stage_a(0)
            stage_g(0)
            for t in range(1, NSLOT):
                stage_a(t)
                stage_b(t - 1)
                stage_g(t)
            stage_b(NSLOT - 1)

            # spk overwrites fs16 in place (elementwise, same AP)
            nc.vector.tensor_tensor(fs16, fs16, linT, ALU.subtract)
            saved[bi] = (batch, blk, fs16, linT)

        def mlp(bi):
            batch, blk, spk16, linT = saved.pop(bi)
            out_b = d["out"][batch].rearrange("(g p) t -> p g t", p=128)
            h1 = wp.tile([128, 4, BLKW], F16, tag="hA", bufs=1)
            for m in range(4):
                pm = psm.tile([128, BLKW], F32, tag="mlp", bufs=2)
                for g in range(KG):
                    nc.tensor.matmul(pm, lhsT=w0[:, g, m * 128:(m + 1) * 128],
                                     rhs=spk16[:, g, :],
                                     start=(g == 0), stop=(g == KG - 1))
                nc.scalar.activation(h1[:, m, :], pm, LRELU,
                                     bias=b0[:, m:m + 1], alpha=0.01)
            h2 = wp.tile([128, 4, BLKW], F16, tag="hB", bufs=1)
            for m in range(4):
                pm = psm.tile([128, BLKW], F32, tag="mlp", bufs=2)
                for g in range(4):
                    nc.tensor.matmul(pm, lhsT=w1[:, g, m * 128:(m + 1) * 128],
                                     rhs=h1[:, g, :],
                                     start=(g == 0), stop=(g == 3))
                nc.scalar.activation(h2[:, m, :], pm, LRELU,
                                     bias=b1[:, m:m + 1], alpha=0.01)
            z = wp.tile([128, 2, BLKW], F16, tag="z", bufs=1)
            for m in range(2):
                pm = psm.tile([128, BLKW], F32, tag="mlp", bufs=2)
                for g in range(4):
                    nc.tensor.matmul(pm, lhsT=w2[:, g, m * 128:(m + 1) * 128],
                                     rhs=h2[:, g, :],
                                     start=(g == 0), stop=(g == 3))
                nc.scalar.activation(z[:, m, :], pm, AF.Identity,
                                     bias=b2[:, m:m + 1])
            d1 = wp.tile([128, 4, BLKW], F16, tag="hA", bufs=1)
            for m in range(4):
                pm = psm.tile([128, BLKW], F32, tag="mlp", bufs=2)
                for g in range(2):
                    nc.tensor.matmul(pm, lhsT=v0[:, g, m * 128:(m + 1) * 128],
                                     rhs=z[:, g, :],
                                     start=(g == 0), stop=(g == 1))
                nc.scalar.activation(d1[:, m, :], pm, LRELU,
                                     bias=c0[:, m:m + 1], alpha=0.01)
            d2 = wp.tile([128, 4, BLKW], F16, tag="hB", bufs=1)
            for m in range(4):
                pm = psm.tile([128, BLKW], F32, tag="mlp", bufs=2)
                for g in range(4):
                    nc.tensor.matmul(pm, lhsT=v1[:, g, m * 128:(m + 1) * 128],
                                     rhs=d1[:, g, :],
                                     start=(g == 0), stop=(g == 3))
                nc.scalar.activation(d2[:, m, :], pm, LRELU,
                                     bias=c1[:, m:m + 1], alpha=0.01)
            for cc in range(KG):
                pm = psm.tile([128, BLKW], F32, tag="mlp", bufs=2)
                for g in range(4):
                    nc.tensor.matmul(pm, lhsT=v2[:, g, cc * 128:(cc + 1) * 128],
                                     rhs=d2[:, g, :],
                                     start=(g == 0), stop=False)
                nc.tensor.matmul(pm, lhsT=c2r[:, cc * 128:(cc + 1) * 128],
                                 rhs=onesrow, start=False, stop=False)
                # += lin (identity matmul folds the codebook row back in)
                nc.tensor.matmul(pm, lhsT=ident16, rhs=linT[:, cc, :],
                                 start=False, stop=True)
                occ = wp.tile([128, BLKW], F32, tag="occ", bufs=2)
                nc.scalar.copy(occ, pm)
                # dispatch from ScalarE's HWDGE: keeps the store off the Sync
                # queue, which is head-of-line blocked by the next block's
                # lin transposes
                nc.scalar.dma_start(
                    out=out_b[:, cc, blk * BLKR:(blk + 1) * BLKR]
                        .rearrange("p (t y) -> p t y", y=NT),
                    in_=occ.rearrange("p (t x) -> p t x", x=SLOT)[:, :, :NT])

        nblocks = BPC * NBLK
        for bi in range(nblocks + 1):
            if bi < nblocks:
                front(bi)
            if bi >= 1:
                mlp(bi - 1)


def build_nc():
    nc = bacc.Bacc("TRN2", target_bir_lowering=False, debug=False,
                   enable_asserts=False, num_devices=NCORES)
    d = {}
    d["feat"] = nc.dram_tensor("feat", (BPC, C, T), F32, kind="ExternalInput").ap()
    d["cT16"] = nc.dram_tensor("cT16", (C, NBINS), F16, kind="ExternalInput").ap()
    d["bias2"] = nc.dram_tensor("bias2", (2, NBINS), F16, kind="ExternalInput").ap()
    d["cen_ext"] = nc.dram_tensor("cen_ext", (NBINS, CE), F32,
                                  kind="ExternalInput").ap()
    for nm, shp in [("w0T", (C, HID)), ("w1T", (HID, HID)), ("w2T", (HID, EMB)),
                    ("v0T", (EMB, HID)), ("v1T", (HID, HID)), ("v2T", (HID, C))]:
        d[nm] = nc.dram_tensor(nm, shp, F16, kind="ExternalInput").ap()
    for nm, n in [("b0", HID), ("b1", HID), ("b2", EMB),
                  ("c0", HID), ("c1", HID)]:
        d[nm] = nc.dram_tensor(nm, (n,), F32, kind="ExternalInput").ap()
    d["c2r"] = nc.dram_tensor("c2r", (1, C), F16, kind="ExternalInput").ap()
    d["out"] = nc.dram_tensor("out", (BPC, C, T), F32, kind="ExternalOutput").ap()

    with tile.TileContext(nc) as tc:
        _build_body(nc, tc, d)
    nc.compile()
    return nc


def _prep_shared(centroid, ew0, eb0, ew1, eb1, ew2, eb2, dw0, db0, dw1, db1,
                 dw2, db2):
    cen = np.asarray(centroid, np.float32)
    c_norm = (cen.astype(np.float64) ** 2).sum(1)
    bias32 = (-0.5 * c_norm).astype(np.float32)
    bias_c = bias32 - np.float32(bias32.mean())
    b_hi = bias_c.astype(np.float16)
    b_lo = (bias_c - b_hi.astype(np.float32)).astype(np.float16)
    cen_ext = np.zeros((NBINS, CE), np.float32)
    cen_ext[:, :C] = cen
    cen_ext[:, C] = bias32
    cen_ext[:, C + 1] = 1.0
    shared = {
        "cT16": np.ascontiguousarray(cen.T).astype(np.float16),
        "bias2": np.stack([b_hi, b_lo]),
        "cen_ext": cen_ext,
        "w0T": np.ascontiguousarray(np.asarray(ew0).T).astype(np.float16),
        "w1T": np.ascontiguousarray(np.asarray(ew1).T).astype(np.float16),
        "w2T": np.ascontiguousarray(np.asarray(ew2).T).astype(np.float16),
        "v0T": np.ascontiguousarray(np.asarray(dw0).T).astype(np.float16),
        "v1T": np.ascontiguousarray(np.asarray(dw1).T).astype(np.float16),
        "v2T": np.ascontiguousarray(np.asarray(dw2).T).astype(np.float16),
        "b0": np.asarray(eb0, np.float32), "b1": np.asarray(eb1, np.float32),
        "b2": np.asarray(eb2, np.float32), "c0": np.asarray(db0, np.float32),
        "c1": np.asarray(db1, np.float32),
        "c2r": np.asarray(db2, np.float32).reshape(1, C).astype(np.float16),
    }
    return shared


def _get_nc():
    if "nc" not in _CACHE:
        _CACHE["nc"] = build_nc()
    return _CACHE["nc"]


def run(inputs, trace=False):
    feature = np.ascontiguousarray(np.asarray(inputs["feature"], np.float32))
    shared = _prep_shared(**{k: v for k, v in inputs.items() if k != "feature"})
    nc = _get_nc()
    in_maps = []
    for c in range(NCORES):
        m = dict(shared)
        m["feat"] = np.ascontiguousarray(feature[c * BPC:(c + 1) * BPC])
        in_maps.append(m)
    kw = {}
    if trace:
        kw = dict(trace=True, trace_cores=list(range(NCORES)))
    res = bass_utils.run_bass_kernel_spmd(nc, in_maps, core_ids=list(range(NCORES)),
                                          **kw)
    out = np.empty((B, C, T), np.float32)
    for c in range(NCORES):
        out[c * BPC:(c + 1) * BPC] = res.results[c]["out"]
    return out, res


def kernel(**inputs) -> np.ndarray:
    out, _ = run(inputs, trace=False)
    return out
